# revision 5
# baseline (speedup 1.0000x reference)
"""Cost-volume concatenation kernel for Trainium2 (8 NeuronCores).

Reference computation:
    out[b, c,    d, h, x] = left [b, c, h, x]          if 0 <= x - disp_d < W else 0
    out[b, C+c,  d, h, x] = right[b, c, h, x - disp_d] if 0 <= x - disp_d < W else 0
with disp_d = d - 112 for d in [0, 128), shapes left/right [1, 32, 128, 256] f32,
output [1, 64, 128, 128, 256] f32 (1 GiB).  Pure data movement -> DMA-only kernel.

Sharding: H is split 16 rows per core, so every core runs the identical program
(all 128 disparities; the column windows are compile-time constants).

Per core the host passes:
  lpad [512, 256]: left rows in (c, h) order
  rpad [512, 384]: [15 zeros | right row | 113 zeros]   (zero-padded right)
so that for every d the right half of the output is a single full-width window
copy  out_right[., d, ., x] = rpad[., x + 127 - d]  -- the zero padding provides
the out-of-range masking for free.  The left half is a window copy of left plus
a zero-fill of the margin, sourced from rpad's zero columns.
"""

import sys

sys.path.insert(0, "/opt/trn_rl_repo")

import numpy as np

import concourse.bass as bass
import concourse.mybir as mybir
from concourse.bass_utils import run_bass_kernel_spmd

F32 = mybir.dt.float32
N_CORES = 8
B, C, H, W = 1, 32, 128, 256
HS = H // N_CORES          # 16 rows of H per core
D = 128                    # disparities; disp = d - 112
ROWS = C * HS              # 512 (c, h) rows per core
RPW = 384                  # rpad row width: 15 zeros + 256 data + 113 zeros
ZCOL = W + 15              # 271: first guaranteed-zero column of rpad (112 wide)

_PROGRAMS = {}


def _build_program(repeat=1):
    """Build the SPMD program. `repeat` re-runs the store pass N times on the
    same output (used by the test harness for differential HW timing)."""
    nc = bass.Bass()
    lpad = nc.declare_dram_parameter("lpad", [ROWS, W], F32, isOutput=False)
    rpad = nc.declare_dram_parameter("rpad", [ROWS, RPW], F32, isOutput=False)
    out = nc.declare_dram_parameter("out", [2 * C, D, HS, W], F32, isOutput=True)

    with (
        nc.sbuf_tensor([128, 4 * W], F32) as lt,
        nc.sbuf_tensor([128, 4 * RPW], F32) as rt,
        nc.semaphore("load_sem") as load_sem,
        nc.semaphore("sp_sem") as sp_sem,
        nc.semaphore("act_sem") as act_sem,
        nc.Block() as block,
    ):
        # SBUF holds 4 consecutive (c, h) rows per partition: row r -> (p, k) =
        # (r // 4, r % 4).  Enumeration order (p, k, x) == (c, h, x) order of the
        # DRAM side, so src/dst DMA streams pair up element-for-element.
        lt3 = lt[:, :].rearrange("p (k x) -> p k x", k=4)
        rt3 = rt[:, :].rearrange("p (k x) -> p k x", k=4)

        @block.sync
        def _(sync):
            sync.dma_start(out=lt[:, :], in_=lpad[:, :]).then_inc(load_sem, 16)
            sync.dma_start(out=rt[:, :], in_=rpad[:, :]).then_inc(load_sem, 16)
            sync.wait_ge(load_sem, 32)
            n = 0
            for _ in range(repeat):
                for d in range(D):
                    # out_right[., d, ., x] = rpad[., x + 127 - d], full width.
                    sync.dma_start(
                        out=out[C : 2 * C, d, :, :],
                        in_=rt3[:, :, 127 - d : 127 - d + W],
                    ).then_inc(sp_sem, 16)
                    n += 1
            sync.wait_ge(sp_sem, 16 * n)

        @block.scalar
        def _(scalar):
            scalar.wait_ge(load_sem, 32)
            n = 0
            for _ in range(repeat):
                for d in range(D):
                    disp = d - 112
                    if disp <= 0:
                        wv = W + disp  # valid columns [0, wv); zeros [wv, W)
                        scalar.dma_start(
                            out=out[0:C, d, :, 0:wv], in_=lt3[:, :, 0:wv]
                        ).then_inc(act_sem, 16)
                        n += 1
                        if disp < 0:
                            with nc.allow_non_contiguous_dma(reason="1-col margin"):
                                scalar.dma_start(
                                    out=out[0:C, d, :, wv:W],
                                    in_=rt3[:, :, ZCOL : ZCOL - disp],
                                ).then_inc(act_sem, 16)
                            n += 1
                    else:
                        # zeros [0, disp); valid columns [disp, W)
                        with nc.allow_non_contiguous_dma(reason="1-col margin"):
                            scalar.dma_start(
                                out=out[0:C, d, :, 0:disp],
                                in_=rt3[:, :, ZCOL : ZCOL + disp],
                            ).then_inc(act_sem, 16)
                        scalar.dma_start(
                            out=out[0:C, d, :, disp:W], in_=lt3[:, :, disp:W]
                        ).then_inc(act_sem, 16)
                        n += 2
            scalar.wait_ge(act_sem, 16 * n)

    return nc


def _get_program(repeat=1):
    if repeat not in _PROGRAMS:
        _PROGRAMS[repeat] = _build_program(repeat)
    return _PROGRAMS[repeat]


def kernel(left, right):
    left = np.asarray(left, dtype=np.float32)
    right = np.asarray(right, dtype=np.float32)
    nc = _get_program()

    in_maps = []
    for i in range(N_CORES):
        h0 = i * HS
        lp = np.ascontiguousarray(left[0, :, h0 : h0 + HS, :]).reshape(ROWS, W)
        rp = np.zeros((ROWS, RPW), dtype=np.float32)
        rp[:, 15 : 15 + W] = right[0, :, h0 : h0 + HS, :].reshape(ROWS, W)
        in_maps.append({"lpad": lp, "rpad": rp})

    res = run_bass_kernel_spmd(nc, in_maps, list(range(N_CORES))).results

    outf = np.empty((B, 2 * C, D, H, W), dtype=np.float32)
    for i in range(N_CORES):
        outf[0, :, :, i * HS : (i + 1) * HS, :] = res[i]["out"]
    return outf


# revision 7
# speedup vs baseline: 2.1441x; 2.1441x over previous
"""Cost-volume concatenation kernel for Trainium2 (8 NeuronCores).

Reference computation:
    out[b, c,    d, h, x] = left [b, c, h, x]          if 0 <= x - disp_d < W else 0
    out[b, C+c,  d, h, x] = right[b, c, h, x - disp_d] if 0 <= x - disp_d < W else 0
with disp_d = d - 112 for d in [0, 128), shapes left/right [1, 32, 128, 256] f32,
output [1, 64, 128, 128, 256] f32 (1 GiB).  Pure data movement -> DMA-only kernel.

Sharding: H is split 16 rows per core, so every core runs the identical program
(all 128 disparities; the column windows are compile-time constants).

Per core the host passes:
  lpad [512, 256]: left rows in (c, h) order
  rpad [512, 384]: [15 zeros | right row | 113 zeros]   (zero-padded right)
so that for every d the right half of the output is a single full-width window
copy  out_right[., d, ., x] = rpad[., x + 127 - d]  -- the zero padding provides
the out-of-range masking for free.  The left half is a window copy of left plus
a zero-fill of the margin, sourced from rpad's zero columns.
"""

import sys

sys.path.insert(0, "/opt/trn_rl_repo")

import numpy as np

import concourse.bass as bass
import concourse.mybir as mybir
from concourse.bass_utils import run_bass_kernel_spmd

F32 = mybir.dt.float32
N_CORES = 8
B, C, H, W = 1, 32, 128, 256
HS = H // N_CORES          # 16 rows of H per core
D = 128                    # disparities; disp = d - 112
ROWS = C * HS              # 512 (c, h) rows per core
RPW = 384                  # rpad row width: 15 zeros + 256 data + 113 zeros
ZCOL = W + 15              # 271: first guaranteed-zero column of rpad (112 wide)

_PROGRAMS = {}


def _build_program(repeat=1):
    """Build the SPMD program. `repeat` re-runs the store pass N times on the
    same output (used by the test harness for differential HW timing)."""
    nc = bass.Bass()
    lpad = nc.declare_dram_parameter("lpad", [ROWS, W], F32, isOutput=False)
    rpad = nc.declare_dram_parameter("rpad", [ROWS, RPW], F32, isOutput=False)
    out = nc.declare_dram_parameter("out", [2 * C, D, HS, W], F32, isOutput=True)

    with (
        nc.sbuf_tensor([128, 4 * W], F32) as lt,
        nc.sbuf_tensor([128, 4 * RPW], F32) as rt,
        nc.semaphore("load_sem") as load_sem,
        nc.semaphore("sp_sem") as sp_sem,
        nc.Block() as block,
    ):
        # SBUF holds 4 consecutive (c, h) rows per partition: row r -> (p, k) =
        # (r // 4, r % 4).  Enumeration order (p, k, x) == (c, h, x) order of the
        # DRAM side, so src/dst DMA streams pair up element-for-element.
        lt3 = lt[:, :].rearrange("p (k x) -> p k x", k=4)
        rt3 = rt[:, :].rearrange("p (k x) -> p k x", k=4)

        # All stores on ONE HWDGE ring (sync): splitting across sync+scalar
        # rings measured 2.1x slower for the same bytes.  The zero margins of
        # the left half are never written: run_bass_kernel_spmd pre-zeros
        # ExternalOutput buffers (documented contract), so skipping them both
        # preserves correctness and avoids tiny-descriptor DMAs.
        @block.sync
        def _(sync):
            sync.dma_start(out=lt[:, :], in_=lpad[:, :]).then_inc(load_sem, 16)
            sync.dma_start(out=rt[:, :], in_=rpad[:, :]).then_inc(load_sem, 16)
            sync.wait_ge(load_sem, 32)
            n = 0
            for _ in range(repeat):
                for d in range(D):
                    disp = d - 112
                    # out_right[., d, ., x] = rpad[., x + 127 - d], full width.
                    sync.dma_start(
                        out=out[C : 2 * C, d, :, :],
                        in_=rt3[:, :, 127 - d : 127 - d + W],
                    ).then_inc(sp_sem, 16)
                    n += 1
                    # out_left: window copy of left; margins stay pre-zeroed.
                    if disp <= 0:
                        wv = W + disp  # valid columns [0, wv)
                        sync.dma_start(
                            out=out[0:C, d, :, 0:wv], in_=lt3[:, :, 0:wv]
                        ).then_inc(sp_sem, 16)
                    else:
                        sync.dma_start(
                            out=out[0:C, d, :, disp:W], in_=lt3[:, :, disp:W]
                        ).then_inc(sp_sem, 16)
                    n += 1
            sync.wait_ge(sp_sem, 16 * n)

    return nc


def _get_program(repeat=1):
    if repeat not in _PROGRAMS:
        _PROGRAMS[repeat] = _build_program(repeat)
    return _PROGRAMS[repeat]


def kernel(left, right):
    left = np.asarray(left, dtype=np.float32)
    right = np.asarray(right, dtype=np.float32)
    nc = _get_program()

    in_maps = []
    for i in range(N_CORES):
        h0 = i * HS
        lp = np.ascontiguousarray(left[0, :, h0 : h0 + HS, :]).reshape(ROWS, W)
        rp = np.zeros((ROWS, RPW), dtype=np.float32)
        rp[:, 15 : 15 + W] = right[0, :, h0 : h0 + HS, :].reshape(ROWS, W)
        in_maps.append({"lpad": lp, "rpad": rp})

    res = run_bass_kernel_spmd(nc, in_maps, list(range(N_CORES))).results

    outf = np.empty((B, 2 * C, D, H, W), dtype=np.float32)
    for i in range(N_CORES):
        outf[0, :, :, i * HS : (i + 1) * HS, :] = res[i]["out"]
    return outf


# revision 9
# speedup vs baseline: 4.8135x; 2.2450x over previous
"""Cost-volume concatenation kernel for Trainium2 (8 NeuronCores).

Reference computation:
    out[b, c,    d, h, x] = left [b, c, h, x]          if 0 <= x - disp_d < W else 0
    out[b, C+c,  d, h, x] = right[b, c, h, x - disp_d] if 0 <= x - disp_d < W else 0
with disp_d = d - 112 for d in [0, 128), shapes left/right [1, 32, 128, 256] f32,
output [1, 64, 128, 128, 256] f32 (1 GiB).  Pure data movement -> DMA-only kernel.

Sharding: H is split 16 rows per core, so every core runs the identical program
(all 128 disparities; the column windows are compile-time constants).

Per core the host passes:
  lpad  [512, 256]: left rows in (c, h) order
  rpad  [512, 384]: [15 zeros | right row | 113 zeros]   (zero-padded right)
  ltail [512, 240]: 15 blocks of 16 cols; block j = [zeros(j+1) | left[j+1:16]]
  lbnd  [512, 896]: 112 blocks of 8 cols; block d = [left[ma:wv) | zeros] where
                    wv = 144+d, ma = wv & ~7  (the ragged 32B-boundary patch)

Measured HW facts this design is built on (verified by probes on this platform):
  * One HWDGE ring (sync) only: splitting stores across sync+scalar rings is
    2.1x slower for the same bytes.
  * HBM-write descriptors must start AND end on 32B boundaries; unaligned
    column windows are ~3-4x slower (RMW on partial AXI beats).  SBUF-read
    alignment does not matter.  Aligned variable-width windows are full speed.
  * run_bass_kernel_spmd pre-zeros ExternalOutput buffers (documented
    contract: "kernels that don't write every element rely on that"), so the
    wide zero margins of the left half are never written at all.

Write plan per disparity d (disp = d - 112):
  right half: one full-width window copy of rpad (dst aligned; the masking
      zeros ride along from the padding).
  left half, disp < 0:  [0, ma) from lpad (aligned), [ma, ma+8) ragged patch
      from lbnd (32B write, mixed data|zeros), [ma+8, W) left pre-zeroed.
  left half, disp == 0: full-width copy of lpad.
  left half, disp > 0:  [0, 16) from ltail (zeros|data), [16, W) from lpad.
  No two writes overlap, so no ordering constraints exist.
"""

import sys

sys.path.insert(0, "/opt/trn_rl_repo")

import numpy as np

import concourse.bass as bass
import concourse.mybir as mybir
from concourse.bass_utils import run_bass_kernel_spmd

F32 = mybir.dt.float32
N_CORES = 8
B, C, H, W = 1, 32, 128, 256
HS = H // N_CORES          # 16 rows of H per core
D = 128                    # disparities; disp = d - 112
ROWS = C * HS              # 512 (c, h) rows per core
RPW = 384                  # rpad row width: 15 zeros + 256 data + 113 zeros
ZCOL = W + 15              # 271: first guaranteed-zero column of rpad
NTAIL = 15                 # positive disparities 1..15 -> ltail blocks
TW = 16                    # ltail block width (64B, aligned)
NB = 112                   # negative disparities -> lbnd blocks
BW = 8                     # lbnd block width (32B, aligned)

_PROGRAMS = {}


def _build_program(repeat=1):
    """Build the SPMD program. `repeat` re-runs the store pass N times on the
    same output (used by the test harness for differential HW timing)."""
    nc = bass.Bass()
    lpad = nc.declare_dram_parameter("lpad", [ROWS, W], F32, isOutput=False)
    rpad = nc.declare_dram_parameter("rpad", [ROWS, RPW], F32, isOutput=False)
    ltail = nc.declare_dram_parameter("ltail", [ROWS, NTAIL * TW], F32, isOutput=False)
    lbnd = nc.declare_dram_parameter("lbnd", [ROWS, NB * BW], F32, isOutput=False)
    out = nc.declare_dram_parameter("out", [2 * C, D, HS, W], F32, isOutput=True)

    with (
        nc.sbuf_tensor([128, 4 * W], F32) as lt,
        nc.sbuf_tensor([128, 4 * RPW], F32) as rt,
        nc.sbuf_tensor([128, 4 * NTAIL * TW], F32) as tl,
        nc.sbuf_tensor([128, 4 * NB * BW], F32) as lb,
        nc.semaphore("load_sem") as load_sem,
        nc.semaphore("sp_sem") as sp_sem,
        nc.Block() as block,
    ):
        # SBUF holds 4 consecutive (c, h) rows per partition: row r -> (p, k) =
        # (r // 4, r % 4).  Enumeration order (p, k, x) == (c, h, x) order of
        # the DRAM side, so src/dst DMA streams pair up element-for-element.
        lt3 = lt[:, :].rearrange("p (k x) -> p k x", k=4)
        rt3 = rt[:, :].rearrange("p (k x) -> p k x", k=4)
        tl3 = tl[:, :].rearrange("p (k x) -> p k x", k=4)
        lb3 = lb[:, :].rearrange("p (k x) -> p k x", k=4)

        @block.sync
        def _(sync):
            sync.dma_start(out=lt[:, :], in_=lpad[:, :]).then_inc(load_sem, 16)
            sync.dma_start(out=rt[:, :], in_=rpad[:, :]).then_inc(load_sem, 16)
            sync.dma_start(out=tl[:, :], in_=ltail[:, :]).then_inc(load_sem, 16)
            sync.dma_start(out=lb[:, :], in_=lbnd[:, :]).then_inc(load_sem, 16)
            sync.wait_ge(load_sem, 64)
            n = 0
            for _ in range(repeat):
                for d in range(D):
                    disp = d - 112
                    # Right half: out_right[., d, ., x] = rpad[., x + 127 - d],
                    # full width (dst aligned; masking zeros from the padding).
                    sync.dma_start(
                        out=out[C : 2 * C, d, :, :],
                        in_=rt3[:, :, 127 - d : 127 - d + W],
                    ).then_inc(sp_sem, 16)
                    n += 1
                    if disp < 0:
                        wv = W + disp          # valid columns [0, wv)
                        ma = wv & ~7           # aligned portion [0, ma)
                        sync.dma_start(
                            out=out[0:C, d, :, 0:ma], in_=lt3[:, :, 0:ma]
                        ).then_inc(sp_sem, 16)
                        n += 1
                        if wv != ma:           # ragged 32B patch [ma, ma+8)
                            sync.dma_start(
                                out=out[0:C, d, :, ma : ma + BW],
                                in_=lb3[:, :, BW * d : BW * (d + 1)],
                            ).then_inc(sp_sem, 16)
                            n += 1
                        # [max(ma+8, wv), W) stays pre-zeroed.
                    elif disp == 0:
                        sync.dma_start(
                            out=out[0:C, d, :, :], in_=lt3[:, :, 0:W]
                        ).then_inc(sp_sem, 16)
                        n += 1
                    else:
                        j = disp - 1           # [0,16) = zeros|data from ltail
                        sync.dma_start(
                            out=out[0:C, d, :, 0:TW],
                            in_=tl3[:, :, TW * j : TW * (j + 1)],
                        ).then_inc(sp_sem, 16)
                        sync.dma_start(
                            out=out[0:C, d, :, TW:W], in_=lt3[:, :, TW:W]
                        ).then_inc(sp_sem, 16)
                        n += 2
            sync.wait_ge(sp_sem, 16 * n)

    return nc


def _get_program(repeat=1):
    if repeat not in _PROGRAMS:
        _PROGRAMS[repeat] = _build_program(repeat)
    return _PROGRAMS[repeat]


def make_in_maps(left, right):
    """Host-side sharding: slice H into per-core row blocks and build the
    padded/derived input tensors."""
    in_maps = []
    for i in range(N_CORES):
        h0 = i * HS
        lrows = np.ascontiguousarray(left[0, :, h0 : h0 + HS, :]).reshape(ROWS, W)
        rp = np.zeros((ROWS, RPW), dtype=np.float32)
        rp[:, 15 : 15 + W] = right[0, :, h0 : h0 + HS, :].reshape(ROWS, W)
        tlb = np.zeros((ROWS, NTAIL * TW), dtype=np.float32)
        for j in range(NTAIL):
            disp = j + 1
            tlb[:, TW * j + disp : TW * (j + 1)] = lrows[:, disp:TW]
        lbb = np.zeros((ROWS, NB * BW), dtype=np.float32)
        for d in range(NB):
            wv = W + (d - 112)  # = 144 + d
            ma = wv & ~7
            lbb[:, BW * d : BW * d + (wv - ma)] = lrows[:, ma:wv]
        in_maps.append({"lpad": lrows, "rpad": rp, "ltail": tlb, "lbnd": lbb})
    return in_maps


def kernel(left, right):
    left = np.asarray(left, dtype=np.float32)
    right = np.asarray(right, dtype=np.float32)
    nc = _get_program()
    in_maps = make_in_maps(left, right)
    res = run_bass_kernel_spmd(nc, in_maps, list(range(N_CORES))).results
    outf = np.empty((B, 2 * C, D, H, W), dtype=np.float32)
    for i in range(N_CORES):
        outf[0, :, :, i * HS : (i + 1) * HS, :] = res[i]["out"]
    return outf
